# revision 1
# baseline (speedup 1.0000x reference)
"""Trainium2 Bass kernel for nn_NeuralMMMModel (MMM: adstock scan + saturation + MLPs).

Key math: the reference's lax.scan over T only feeds its LAST carry downstream:
    last_ad[b, c] = sum_t d[c]^(T-1-t) * x[b, t, c],   d = sigmoid(decay) < 1.
Old timesteps decay geometrically, so steps whose weight falls below ~1e-8
contribute nothing representable in fp32; we truncate to the last K steps,
choosing K at runtime from the actual decay/beta/|x| values (K == T when decay
is close to 1).

Device layout: channels on partitions (C=128), batch-major free dim [b][t].
The recurrence runs as DVE tensor_tensor_scan ops (state = d*state + x along
the free dim, fp32 carry — same association order as the reference's scan).
The scan chains across b-boundaries, but the leaked carry is attenuated by
d^K <= ~1e-8 — below fp32 noise by construction of K.

The whole kernel uses ONE ACT table set (sigmoid_and_others: sigmoid, erf,
identity), so there are no mid-kernel ACT table reloads:
  - saturation: r = 1/sigmoid(bcl*last_ad) = 1 + exp(-bcl*last_ad), with the
    extra 1 folded into the next layer's bias on the host;
  - exact gelu via erf: 2*gelu(u) = u*(1+erf(u/sqrt2)), with the 0.5 folded
    into the next layer's weights on the host.
The epilogue runs feature-on-partition (biases are per-partition scalars),
sliced over geometrically shrinking batch ranges so all but the last tiny
slice overlap the DMA/scan phase. The control-vars Linear is folded into the
output net on the host (Wc @ Wo1[128:160]); dummy bf16 matmuls keep the PE
HAM monitor warm so the fp32 epilogue matmuls run at 2.4 GHz.

Sharding: pure data parallelism, batch B=2048 split across 8 cores (256 each).
"""

import contextlib
import numpy as np
from contextlib import ExitStack

import concourse.bass as bass
import concourse.tile as tile
from concourse import mybir, bacc
from concourse.bass_utils import run_bass_kernel_spmd

B, T, C, NCTRL = 2048, 512, 128, 10
NCORES = 8
BS = B // NCORES          # 256 batch rows per core
HID = 2 * C               # 256
HO = 64

F32 = mybir.dt.float32
RSQ2 = 0.7071067811865476
WARM = 2                    # PE warm-up matmuls per scan chunk
XBUFS = 5                   # x-tile double-buffering depth
SBUFS = 5                   # scan-output buffering depth

# Params tile column offsets (K-independent).
O_DCOL = 0                  # [128, 1]   d[c]
O_BCL = 1                   # [128, 1]   max(beta, 0.01)[c]
O_W1N = 2                   # [128, 256] -(W1 * 2*sigmoid(alpha))
O_W2S = O_W1N + 256         # [128, 256] 0.5*W2 row-chunks (two 128-wide lhsT)
O_WO1A = O_W2S + 256        # [128, 64]  Wo1[:128, :]
O_WCOMBO = O_WO1A + HO      # [128, 64]  rows 0:10 = Wc @ Wo1[128:160]
O_WO2 = O_WCOMBO + HO       # [128, 1]   rows 0:64 = 0.5*Wo2[:, 0]
O_B1P = O_WO2 + 1           # 2 cols     b1 + 2*colsum(W1*a2), split 128/128
O_BO1P = O_B1P + 2          # 1 col      rows 0:64
PW = O_BO1P + 1             # params width (control-vars ship separately)

_kernel_cache: dict[int, object] = {}


def _pick_chunks(K: int) -> list[int]:
    """Batch rows per scan chunk: big DMAs win; keep x+scan pools (10 bufs)
    within ~150KB/partition of SBUF."""
    ch = 64
    while ch > 4 and ch * K * 4 > 15 * 1024:
        ch //= 2
    while BS % ch:
        ch //= 2
    ch = max(ch, 1)
    sizes = [ch] * (BS // ch)
    assert sum(sizes) == BS
    return sizes


def _group_slices(chunks: list[int]) -> list[tuple[int, int]]:
    """Epilogue batch slices: two halves measured fastest on HW (fewer
    cross-engine hops than finer slicing, some overlap with the scan phase)."""
    total = sum(chunks)
    h = 0
    i = 0
    while i < len(chunks) and h < total // 2:
        h += chunks[i]
        i += 1
    return [(0, h), (h, total - h)]


def _build(K: int, reps: int = 1, mode: str = "full"):
    """Build + compile the Bass program for truncation length K.

    reps > 1 wraps the whole compute body in a hardware For_i loop
    (re-reading the same inputs); used only for steady-state HW timing."""
    chunks = _pick_chunks(K)
    CH = chunks[0]
    slices = _group_slices(chunks)
    wmax = max(w for _, w in slices)
    nc = bacc.Bacc("TRN2", target_bir_lowering=False, debug=False,
                   num_devices=NCORES)
    xt = nc.dram_tensor("xt", [C, BS * K], F32, kind="ExternalInput")
    params = nc.dram_tensor("params", [128, PW], F32, kind="ExternalInput")
    cvt_in = nc.dram_tensor("cvt", [NCTRL, BS], F32, kind="ExternalInput")
    y_out = nc.dram_tensor("y", [1, BS], F32, kind="ExternalOutput")

    with tile.TileContext(nc) as tc, ExitStack() as ctx:
        const = ctx.enter_context(tc.tile_pool(name="const", bufs=1))
        xpool = ctx.enter_context(tc.tile_pool(name="x", bufs=XBUFS))
        spool = ctx.enter_context(tc.tile_pool(name="scan", bufs=SBUFS))
        work = ctx.enter_context(tc.tile_pool(name="work", bufs=1))
        epool = ctx.enter_context(tc.tile_pool(name="epi", bufs=2))
        psum = ctx.enter_context(tc.tile_pool(name="psum", bufs=1, space="PSUM"))

        # Params go via SWDGE (gpsimd) so the x stream owns the HWDGE queue
        # from the first cycle.
        par = const.tile([128, PW], F32)
        nc.gpsimd.dma_start(out=par, in_=params[:, :])
        cvt = const.tile([128, BS], F32)
        nc.gpsimd.memset(cvt[:, :], 0.0)
        nc.gpsimd.dma_start(out=cvt[0:NCTRL, :], in_=cvt_in[:, :])

        # ---- adstock scan + saturation: r = 1 + exp(-bcl*last_ad) ----
        bcl = par[:, O_BCL:O_BCL + 1]
        warm_ps = psum.tile([1, 512], F32)
        # Materialized per-partition d replicated along free dim (a stride-1
        # data0 for the scan; a zero-stride broadcast AP may deoptimize it).
        d_rep = const.tile([128, CH * K], F32)
        nc.vector.memset(d_rep, 1.0)
        nc.vector.tensor_scalar_mul(out=d_rep, in0=d_rep,
                                    scalar1=par[:, O_DCOL:O_DCOL + 1])
        with (tc.For_i(0, reps, 1) if reps > 1 else contextlib.nullcontext()):
         r = work.tile([128, BS], F32, tag="r", name="r")
         b0 = 0
         for ch in chunks:
             xg = xpool.tile([128, CH * K], F32, tag="xg", name="xg")
             nc.sync.dma_start(out=xg[:, :ch * K],
                               in_=xt[:, b0 * K:(b0 + ch) * K])
             if mode == "dma":
                 b0 += ch
                 continue
             sg = spool.tile([128, CH * K], F32, tag="sg", name="sg")
             nc.vector.tensor_tensor_scan(
                 out=sg[:, :ch * K], data0=d_rep[:, :ch * K], data1=xg[:, :ch * K],
                 initial=0.0,
                 op0=mybir.AluOpType.mult, op1=mybir.AluOpType.add)
             if mode == "rawscan":
                 b0 += ch
                 continue
             nc.scalar.activation(
                 out=r[:, b0:b0 + ch], in_=sg[:, K - 1:ch * K:K],
                 func=mybir.ActivationFunctionType.Exp, scale=bcl)
             # Dummy bf16 matmuls chained to scan outputs keep the PE warm.
             wn = min(256, ch * K)
             wsrc = sg[:, 0:wn].bitcast(mybir.dt.bfloat16)
             for _ in range(WARM):
                 nc.tensor.matmul(warm_ps[:, 0:2 * wn], lhsT=wsrc[:, 0:1],
                                  rhs=wsrc[:, 0:2 * wn])
             b0 += ch

         if mode in ("dma", "scan", "rawscan"):
             nc.sync.dma_start(out=y_out[:, :], in_=par[0:1, 0:BS])
             continue_epilogue = False
         else:
             continue_epilogue = True

         def gelu1(pres, o_bias, out_ap, parts):
             nc.scalar.activation(out=out_ap, in_=pres,
                                  func=mybir.ActivationFunctionType.Gelu,
                                  bias=par[0:parts, o_bias:o_bias + 1])

         # ---- epilogue over shrinking batch slices ----
         for b0, w in (slices if continue_epilogue else []):
             rh = r[:, b0:b0 + w]

             # h = 2*gelu(b1p2 - (W1*a2).T @ r)
             hp0 = psum.tile([128, wmax], F32, tag="hp0", name="hp0")[:, :w]
             hp1 = psum.tile([128, wmax], F32, tag="hp1", name="hp1")[:, :w]
             nc.tensor.matmul(hp0, lhsT=par[:, O_W1N:O_W1N + 128], rhs=rh)
             nc.tensor.matmul(hp1, lhsT=par[:, O_W1N + 128:O_W1N + 256], rhs=rh)
             h0 = epool.tile([128, wmax], F32, tag="h0", name="h0")[:, :w]
             h1 = epool.tile([128, wmax], F32, tag="h1", name="h1")[:, :w]
             gelu1(hp0, O_B1P, h0, 128)
             gelu1(hp1, O_B1P + 1, h1, 128)

             # interactions (0.5*W2 folded on host; b2 folded into bo1p)
             ip = psum.tile([128, wmax], F32, tag="ip", name="ip")[:, :w]
             nc.tensor.matmul(ip, lhsT=par[:, O_W2S:O_W2S + 128], rhs=h0,
                              start=True, stop=False)
             nc.tensor.matmul(ip, lhsT=par[:, O_W2S + 128:O_W2S + 256], rhs=h1,
                              start=False, stop=True)
             isb = epool.tile([128, wmax], F32, tag="isb", name="isb")[:, :w]
             nc.scalar.activation(out=isb, in_=ip,
                                  func=mybir.ActivationFunctionType.Identity,
                                  bias=0.0)

             # o1 = 2*gelu(Wo1[:128].T @ interactions + Wcombo.T @ cv + bo1p)
             op = psum.tile([HO, wmax], F32, tag="op", name="op")[:, :w]
             nc.tensor.matmul(op, lhsT=par[:, O_WO1A:O_WO1A + HO], rhs=isb,
                              start=True, stop=False)
             nc.tensor.matmul(op, lhsT=par[:, O_WCOMBO:O_WCOMBO + HO],
                              rhs=cvt[:, b0:b0 + w],
                              start=False, stop=True)
             o1 = epool.tile([128, wmax], F32, tag="o1", name="o1")
             nc.gpsimd.memset(o1[HO:128, :], 0.0)
             gelu1(op, O_BO1P, o1[0:HO, :w], HO)

             # y = (0.5*Wo2).T @ o1  (bo2 added on host)
             yp = psum.tile([1, wmax], F32, tag="yp", name="yp")[:, :w]
             nc.tensor.matmul(yp, lhsT=par[:, O_WO2:O_WO2 + 1], rhs=o1[:, :w])
             ysb = epool.tile([1, wmax], F32, tag="ysb", name="ysb")[:, :w]
             nc.scalar.activation(out=ysb, in_=yp,
                                  func=mybir.ActivationFunctionType.Identity,
                                  bias=0.0)
             nc.sync.dma_start(out=y_out[:, b0:b0 + w], in_=ysb)

    nc.compile()
    return nc


def _pick_K(d64, bcl64, maxabs):
    """Smallest K <= T whose truncated tail is < 3e-7 in z = bcl*last_ad."""
    d_max = float(d64.max())
    if d_max >= 1.0 - 1e-12:
        return T
    bcl_max = float(bcl64.max())
    scale = max(bcl_max * max(maxabs, 1e-30) / (1.0 - d_max), 1e-30)
    k = np.log(3e-7 / scale) / np.log(d_max)  # d_max^K * scale <= 3e-7
    return max(min(T, int(np.ceil(max(k, 1.0)))), 4)


def kernel(channel_spend, control_vars, decay, alpha, beta,
           W1, b1, W2, b2, Wc, bc, Wo1, bo1, Wo2, bo2):
    x = np.asarray(channel_spend, dtype=np.float32)
    cv = np.asarray(control_vars, dtype=np.float32)
    decay = np.asarray(decay, dtype=np.float64)
    alpha = np.asarray(alpha, dtype=np.float64)
    beta = np.asarray(beta, dtype=np.float64)
    W1 = np.asarray(W1, dtype=np.float64)
    b1 = np.asarray(b1, dtype=np.float64)
    W2 = np.asarray(W2, dtype=np.float32)
    b2 = np.asarray(b2, dtype=np.float64)
    Wc = np.asarray(Wc, dtype=np.float64)
    bc = np.asarray(bc, dtype=np.float64)
    Wo1 = np.asarray(Wo1, dtype=np.float64)
    bo1 = np.asarray(bo1, dtype=np.float64)
    Wo2 = np.asarray(Wo2, dtype=np.float32)
    bo2 = np.asarray(bo2, dtype=np.float64)

    d64 = 1.0 / (1.0 + np.exp(-decay))
    a64 = 2.0 / (1.0 + np.exp(-alpha))
    bcl64 = np.maximum(beta, 0.01)

    maxabs = max(abs(float(x.max())), abs(float(x.min())))
    K = _pick_K(d64, bcl64, maxabs)

    W1a = W1 * a64[:, None]                       # [C, 2C]
    wcombo = (Wc @ Wo1[128:128 + 32]).astype(np.float32)     # [10, 64]
    # h_pre = b1 + colsum(W1a) - W1a.T @ e,  e = exp(-bcl*last_ad)
    b1p = (b1 + W1a.sum(axis=0)).astype(np.float32)          # [2C]
    bo1p = (bo1 + b2 @ Wo1[:128] + bc @ Wo1[128:128 + 32]).astype(np.float32)
    bo2f = float(bo2.reshape(-1)[0])

    par_base = np.zeros((128, PW), dtype=np.float32)
    par_base[:, O_DCOL] = d64.astype(np.float32)
    par_base[:, O_BCL] = (-bcl64).astype(np.float32)
    par_base[:, O_W1N:O_W1N + 256] = (-W1a).astype(np.float32)
    par_base[:, O_W2S:O_W2S + 128] = W2[0:128, :]
    par_base[:, O_W2S + 128:O_W2S + 256] = W2[128:256, :]
    par_base[:, O_WO1A:O_WO1A + HO] = Wo1[:128, :].astype(np.float32)
    par_base[0:NCTRL, O_WCOMBO:O_WCOMBO + HO] = wcombo
    par_base[0:HO, O_WO2] = Wo2[:, 0]
    par_base[:, O_B1P] = b1p[:128]
    par_base[:, O_B1P + 1] = b1p[128:]
    par_base[0:HO, O_BO1P] = bo1p

    in_maps = []
    for i in range(NCORES):
        xs = x[i * BS:(i + 1) * BS, T - K:, :]            # [BS, K, C]
        xti = np.ascontiguousarray(xs.transpose(2, 0, 1))  # [C, BS, K]
        cvt_i = np.ascontiguousarray(cv[i * BS:(i + 1) * BS, :].T)
        in_maps.append({"xt": xti.reshape(C, BS * K),
                        "params": par_base, "cvt": cvt_i})

    nc = _kernel_cache.get(K)
    if nc is None:
        nc = _build(K)
        _kernel_cache[K] = nc

    res = run_bass_kernel_spmd(nc, in_maps, core_ids=list(range(NCORES)))
    y = np.concatenate([r["y"].reshape(-1) for r in res.results])
    return (y + np.float32(bo2f)).astype(np.float32)

